# revision 1
# baseline (speedup 1.0000x reference)
"""EmbeddingBag(mean) over ragged char bags on 8 Trainium2 NeuronCores.

Problem: chars [1024, 256, 16] int32 (vocab 256), lengths [1024, 256] int32
in [1, 16], emb_table [256, 50] f32. Output [1024, 256, 50] f32 =
mean(emb_table[chars[b, s, :lengths[b, s]]]) per bag.

Strategy (data-parallel over batch, 128 batch rows -> 32768 bags per core).
Per 128-bag subtile:

  1. Mask pad slots to an out-of-range sentinel (bf16 ops, per 2048-bag
     macro tile).
  2. Broadcast each slot's char column to all 128 partitions via PE
     transposes of free-dim-stride-0 column views -> bf16 [class_part,
     slot, bag] slab in PSUM; evict to SBUF split across ScalarE and
     DVE (GPSIMD has no PSUM access on TRN2).
  3. Two fused tensor_scalar is_equal ops (per-partition scalar = class
     iota, chunk0 = p, chunk1 = p+128) over the whole slab -> one-hot
     slabs at DVE 4x mode.
  4. No count matrices: each one-hot slab slice [class, bag] is used
     directly as a (free-to-load) matmul stationary against the bf16
     embedding chunk [class, 50]; PSUM accumulation over the 32
     (slot, chunk) matmuls produces out[bag, 50] = sum of embeddings.
     Moving size is only 50 rows per matmul.
  5. ScalarE evicts PSUM scaled by 1/length (per-partition scale);
     output DMA is batched per macro tile.
"""

import os
import sys

sys.path.insert(0, "/opt/trn_rl_repo")
sys.path.insert(0, os.path.dirname(os.path.abspath(__file__)))

import numpy as np

import concourse.bacc as bacc
import concourse.bass as bass
from concourse import mybir
from concourse.bass_utils import run_bass_kernel_spmd
from concourse.masks import make_identity
import concourse.tile as tile

B, S, W = 1024, 256, 16
NB_CLASSES = 256
EMB = 50
N_CORES = 8

ROWS_PER_CORE = B // N_CORES          # 128 batch rows
TOK = ROWS_PER_CORE * S               # 32768 bags per core
SUBT = 16                             # 128-bag subtiles per macro tile
MACRO = TOK // (128 * SUBT)           # 64 macro tiles
SENTINEL = 256.0  # masked chars become 256: exact in bf16, matches no class

# Slab eviction split (slots out of 16): ScalarE takes the first
# ACT_SLOTS, GPSIMD the rest. DVE does only the compares.
ACT_SLOTS = int(os.environ.get("K_ACT_SLOTS", "13"))
DEPTH = int(os.environ.get("K_DEPTH", "4"))
BACK_FIRST = int(os.environ.get("K_BACK_FIRST", "1"))
BC_BUFS = int(os.environ.get("K_BC_BUFS", "2"))
OUT_BUFS = int(os.environ.get("K_OUT_BUFS", "2"))
OH_BUFS = int(os.environ.get("K_OH_BUFS", "6"))
BCS_BUFS = int(os.environ.get("K_BCS_BUFS", "4"))

f32 = mybir.dt.float32
bf16 = mybir.dt.bfloat16
i32 = mybir.dt.int32
AF = mybir.ActivationFunctionType
ALU = mybir.AluOpType


def build_program(loop_n: int | None = None) -> bass.Bass:
    nc = bacc.Bacc()
    chars_d = nc.declare_dram_parameter("chars", [TOK, W], i32, isOutput=False)
    len_d = nc.declare_dram_parameter("lengths", [TOK], i32, isOutput=False)
    emb_d = nc.declare_dram_parameter("emb", [NB_CLASSES, EMB], f32, isOutput=False)
    out_d = nc.declare_dram_parameter("out", [TOK, EMB], f32, isOutput=True)

    chars_v = chars_d.rearrange("(mm s p) w -> mm p s w", s=SUBT, p=128)
    len_v = len_d.rearrange("(k p) -> p k", p=128)
    out_v = out_d.rearrange("(mm s p) e -> mm p s e", s=SUBT, p=128)

    with tile.TileContext(nc) as tc:
        with (
            tc.tile_pool(name="singles", bufs=1) as singles,
            tc.tile_pool(name="chars", bufs=2) as chars_pool,
            tc.tile_pool(name="mask", bufs=2) as mask_pool,
            tc.tile_pool(name="bcs", bufs=BCS_BUFS) as bcs_pool,
            tc.tile_pool(name="oh", bufs=OH_BUFS) as oh_pool,
            tc.tile_pool(name="osb", bufs=2) as osb_pool,
            tc.tile_pool(name="bc_ps", bufs=BC_BUFS, space="PSUM") as bc_ps,
            tc.tile_pool(name="out_ps", bufs=OUT_BUFS, space="PSUM") as out_ps,
        ):
            # ---- one-time constants ----
            ident = singles.tile([128, 128], bf16)
            make_identity(nc, ident)

            # per-partition class indices for the two chunks
            iota_p_i = singles.tile([128, 2], i32)
            nc.gpsimd.iota(iota_p_i, pattern=[[128, 2]], channel_multiplier=1)
            iota_p = singles.tile([128, 2], f32)
            nc.vector.tensor_copy(iota_p, iota_p_i)   # col j: p + 128*j

            # slot index (free dim) for mask compare, bf16
            iota_w_i = singles.tile([128, SUBT, W], i32)
            nc.gpsimd.iota(iota_w_i, pattern=[[0, SUBT], [1, W]], channel_multiplier=0)
            iota_w = singles.tile([128, SUBT, W], bf16)
            nc.vector.tensor_copy(iota_w, iota_w_i)

            # embedding chunks in bf16 (counts are exact small ints; the
            # only rounding is the bf16 table quantization, ~0.4% rel)
            emb_f = singles.tile([128, 2, EMB], f32)
            nc.sync.dma_start(out=emb_f[:, 0, :], in_=emb_d[0:128, :])
            nc.sync.dma_start(out=emb_f[:, 1, :], in_=emb_d[128:256, :])
            emb_b = singles.tile([128, 2, EMB], bf16)
            nc.vector.tensor_copy(emb_b, emb_f)

            # lengths -> f32, bf16 and reciprocal for all 256 column-tiles
            len_i = singles.tile([128, TOK // 128], i32)
            nc.sync.dma_start(out=len_i, in_=len_v)
            len_f = singles.tile([128, TOK // 128], f32)
            nc.vector.tensor_copy(len_f, len_i)
            len_b = singles.tile([128, TOK // 128], bf16)
            nc.vector.tensor_copy(len_b, len_f)
            inv_l = singles.tile([128, TOK // 128], f32)
            nc.vector.reciprocal(inv_l, len_f)

            import contextlib
            loop_cm = (
                tc.For_i(0, loop_n, 1) if loop_n is not None
                else contextlib.nullcontext()
            )
            with loop_cm:
                _main_loop(
                    nc, chars_pool, mask_pool, bcs_pool, oh_pool, osb_pool,
                    bc_ps, out_ps, chars_v, out_v, len_b, iota_w, iota_p,
                    ident, emb_b, inv_l,
                )

    nc.finalize()
    return nc


def _main_loop(nc, chars_pool, mask_pool, bcs_pool, oh_pool, osb_pool,
               bc_ps, out_ps, chars_v, out_v, len_b, iota_w, iota_p,
               ident, emb_b, inv_l):
    # Software pipeline, 2 subtiles deep: the front stage (transpose ->
    # evict -> compare) for subtile k runs while the PE issues the
    # accumulation matmuls for subtile k-2, so the PE never waits on the
    # current subtile's evict+compare latency.
    NTILES = MACRO * SUBT
    pending = {}   # k -> (oh tile, osb tile)
    osb = None

    def front(k):
        nonlocal osb
        mm, s = divmod(k, SUBT)
        if s == 0:
            chars_i = chars_pool.tile([128, SUBT, W], i32, tag="chars")
            nc.sync.dma_start(out=chars_i, in_=chars_v[mm])

            # chars -> bf16 (values <= 255, exact)
            cb = mask_pool.tile([128, SUBT, W], bf16, tag="cb")
            nc.vector.tensor_copy(cb, chars_i)
            # m = (iota_w < len); len broadcast over W via step-0 AP
            lrep = bass.AP(
                tensor=len_b.tensor,
                offset=len_b.offset + mm * SUBT,
                ap=[len_b.ap[0], [1, SUBT], [0, W]],
            )
            m = mask_pool.tile([128, SUBT, W], bf16, tag="m")
            nc.vector.tensor_tensor(out=m, in0=iota_w, in1=lrep, op=ALU.is_lt)
            # cmb = (cb - SENTINEL) * m + SENTINEL  (bf16-exact integer ops)
            t1 = mask_pool.tile([128, SUBT, W], bf16, tag="t1")
            nc.vector.scalar_tensor_tensor(
                out=t1, in0=cb, scalar=-SENTINEL, in1=m,
                op0=ALU.add, op1=ALU.mult,
            )
            cmb = mask_pool.tile([128, SUBT, W], bf16, tag="cmb")
            nc.vector.tensor_scalar_add(cmb, t1, SENTINEL)
            front.cmb = cmb
        cmb = front.cmb

        # chars broadcast: per slot, transpose a free-step-0 column view
        # of cmb -> [class_part, bag] slab (bf16 in PSUM)
        bc_p = bc_ps.tile([128, W, 128], bf16, tag="bc")
        for sl in range(W):
            col_rep = bass.AP(
                tensor=cmb.tensor,
                offset=cmb.offset + s * W + sl,
                ap=[cmb.ap[0], [0, 128]],
            )
            nc.tensor.transpose(bc_p[:, sl, :], col_rep, ident)
        # evict broadcast slab, split across ACT and GPSIMD
        bcsb = bcs_pool.tile([128, W, 128], bf16, tag="bcsb")
        nc.scalar.copy(bcsb[:, 0:ACT_SLOTS, :], bc_p[:, 0:ACT_SLOTS, :])
        nc.vector.tensor_copy(bcsb[:, ACT_SLOTS:W, :], bc_p[:, ACT_SLOTS:W, :])

        # one-hot slabs per class chunk (DVE 4x fused compares)
        oh = oh_pool.tile([128, 2, W, 128], bf16, tag="oh")
        for c in range(2):
            nc.vector.tensor_scalar(
                out=oh[:, c, :, :], in0=bcsb,
                scalar1=iota_p[:, c : c + 1], scalar2=None,
                op0=ALU.is_equal,
            )
        pending[k] = oh

    def back(k):
        nonlocal osb
        mm, s = divmod(k, SUBT)
        if s == 0:
            osb = osb_pool.tile([128, SUBT, EMB], f32, tag="osb")
            back.osb_by_mm = getattr(back, "osb_by_mm", {})
            back.osb_by_mm[mm] = osb
        oh = pending.pop(k)
        # out[bag, e] += oh_slab[class, bag]^T @ emb_chunk[class, e]
        # (stationary loads are free; moving size is 50 rows each)
        po = out_ps.tile([128, EMB], f32)
        n_mm = 0
        for c in range(2):
            for sl in range(W):
                nc.tensor.matmul(
                    po,
                    lhsT=oh[:, c, sl, :],
                    rhs=emb_b[:, c, :],
                    start=(n_mm == 0),
                    stop=(n_mm == 2 * W - 1),
                )
                n_mm += 1

        cur = back.osb_by_mm[mm]
        nc.scalar.activation(
            cur[:, s, :], po, AF.Copy, scale=inv_l[:, k : k + 1]
        )
        if s == SUBT - 1:
            nc.sync.dma_start(out=out_v[mm], in_=cur)
            del back.osb_by_mm[mm]

    for k in range(NTILES + DEPTH):
        if BACK_FIRST:
            if k - DEPTH >= 0:
                back(k - DEPTH)
            if k < NTILES:
                front(k)
        else:
            if k < NTILES:
                front(k)
            if k - DEPTH >= 0:
                back(k - DEPTH)


_PROGRAM = None


def _get_program() -> bass.Bass:
    global _PROGRAM
    if _PROGRAM is None:
        _PROGRAM = build_program()
    return _PROGRAM


def run_on_hw(chars, lengths, emb_table, trace=False, **kw):
    nc = _get_program()
    in_maps = []
    for i in range(N_CORES):
        sl = slice(i * ROWS_PER_CORE, (i + 1) * ROWS_PER_CORE)
        in_maps.append(
            {
                "chars": np.ascontiguousarray(chars[sl].reshape(TOK, W)),
                "lengths": np.ascontiguousarray(lengths[sl].reshape(TOK)),
                "emb": np.ascontiguousarray(emb_table),
            }
        )
    res = run_bass_kernel_spmd(nc, in_maps, list(range(N_CORES)), trace=trace, **kw)
    out = np.concatenate(
        [res.results[i]["out"].reshape(ROWS_PER_CORE, S, EMB) for i in range(N_CORES)],
        axis=0,
    )
    return out, res


def kernel(chars, lengths, emb_table):
    out, _ = run_on_hw(chars, lengths, emb_table)
    return out

